# revision 3
# baseline (speedup 1.0000x reference)
"""Trainium2 Bass kernel for nn_EnsembleModel (histogram_binning).

Math:
  hist[p,q]  = sum_{b,i,j} [adds[b,i]==p] * a_arc[b,i,j] * [adds[b,j]==q]
  score      = sigmoid(hist)                                  # [50,50]
  out[b,i,j] = s_arc[b,i,j] + ALPHA * score[pos[b,i], pos[b,j]]

Both the histogram and the gather-broadcast are TensorEngine matmuls against
one-hot matrices (U = onehot(adds) in fp8, VT = onehot(pos).T in bf16)
prepared host-side in partition-major layout:

  phase 1 (per batch):  P[p,jblk] = sum_i U[i,p] A[i,j]   (lhsT=U, rhs=A, N=512)
                        PT chunks = PE-transpose of P
                        hist     += PT.T @ U              (lhsT=PT, rhs=U)
  AllGather(hist shards) over 8 cores -> local DVE tree-sum -> sigmoid*ALPHA
  phase 2 (per batch):  GT[q,i] = sum_p S'[p,q] VT[p,i]   (lhsT=S', rhs=VT)
                        o_psum  = GT.T @ VT  (+ eye128.T @ s accumulated in PSUM)
                        copy o_psum -> SBUF bf16 (alternating DVE / ACT)

All compute in bf16 (a_arc/s_arc rounded on host; one-hot operands exact).
AllGather+local-sum replaces AllReduce: the 8-core AG latency floor (~5us)
is far below the AR mesh path (~40us observed).

Data-parallel over batch: 8 batches per core on 8 NeuronCores.
a-loads + out-stores ride the SP HWDGE ring; s-loads ride the ACT ring,
throttled behind a-loads so the histogram (and the collective behind it)
completes while s is still streaming.
"""

import numpy as np
import ml_dtypes

ALPHA = 0.3
NP = 50          # n_pos
SL = 1024        # sequence length
BZ = 64          # global batch
NCORES = 8
B = BZ // NCORES  # local batch per core
NCH = SL // 128   # 128-row chunks per matrix
NBLK = SL // 512  # 512-col blocks per matrix

_CACHE = {}


def _build_nc():
    import concourse.bacc as bacc
    import concourse.mybir as mybir
    import concourse.tile as tile
    from concourse.tile import add_dep_helper

    f32 = mybir.dt.float32
    bf16 = mybir.dt.bfloat16
    fp8 = mybir.dt.float8e4
    nc = bacc.Bacc(
        "TRN2", target_bir_lowering=False, debug=False, num_devices=NCORES
    )

    a_d = nc.dram_tensor("a", [B, SL, SL], bf16, kind="ExternalInput")
    s_d = nc.dram_tensor("s", [B, SL, SL], bf16, kind="ExternalInput")
    u_d = nc.dram_tensor("u", [128, B, NCH, NP], fp8, kind="ExternalInput")
    vt_d = nc.dram_tensor("vt", [NP, B, SL], bf16, kind="ExternalInput")
    eye_d = nc.dram_tensor("eye", [NP, NP], bf16, kind="ExternalInput")
    eye128_d = nc.dram_tensor("eye128", [128, 128], bf16, kind="ExternalInput")
    out_d = nc.dram_tensor("out", [B, SL, SL], bf16, kind="ExternalOutput")

    with tile.TileContext(nc) as tc:
        with (
            tc.tile_pool(name="const", bufs=1) as const_pool,
            tc.tile_pool(name="apool", bufs=2) as a_pool,
            tc.tile_pool(name="opool", bufs=2) as o_pool,
            tc.tile_pool(name="ppool", bufs=2) as p_pool,
            tc.tile_pool(name="ptsb", bufs=4) as pt_pool,
            tc.tile_pool(name="gtsb", bufs=2) as gt_pool,
            tc.tile_pool(name="small", bufs=1) as small_pool,
            tc.tile_pool(name="dram", bufs=1, space="DRAM") as dram_pool,
        ):
            # Persistent operands — partition-major, one dense DMA each.
            u_sb = const_pool.tile([128, B, NCH, NP], fp8)
            eye_sb = const_pool.tile([NP, NP], bf16)
            eye128_sb = const_pool.tile([128, 128], bf16)
            vt_sb = const_pool.tile([NP, B, SL], bf16)
            s_sb = const_pool.tile([128, B, NCH, SL], bf16)
            nc.sync.dma_start(eye_sb[:], eye_d[:])
            nc.sync.dma_start(eye128_sb[:], eye128_d[:])
            nc.sync.dma_start(u_sb[:], u_d[:])
            nc.sync.dma_start(vt_sb[:], vt_d[:])

            a_loads = []

            # ---- Phase 1: local histogram (PE), a-loads on SP ring ----
            with (
                tc.tile_pool(name="histps", bufs=1, space="PSUM") as hist_pool,
                tc.tile_pool(name="pps", bufs=2, space="PSUM") as pps_pool,
                tc.tile_pool(name="tpps", bufs=2, space="PSUM") as tpps_pool,
            ):
                hist_ps = hist_pool.tile([NP, NP], f32)
                for b in range(B):
                    at = a_pool.tile([128, NCH, SL], bf16, tag="a")
                    ld = nc.sync.dma_start(
                        at[:], a_d[b].rearrange("(c p) j -> p c j", p=128)
                    )
                    a_loads.append(ld)
                    p_sb = p_pool.tile([NP, SL], bf16, tag="p")
                    for jb in range(NBLK):
                        p_ps = pps_pool.tile([NP, 512], f32, tag="pp")
                        for ic in range(NCH):
                            nc.tensor.matmul(
                                p_ps[:],
                                u_sb[:, b, ic, :],
                                at[:, ic, jb * 512:(jb + 1) * 512],
                                start=(ic == 0),
                                stop=(ic == NCH - 1),
                            )
                        nc.vector.tensor_copy(
                            p_sb[:, jb * 512:(jb + 1) * 512], p_ps[:]
                        )
                    # hist += PT.T @ U per 128-chunk of j.
                    for jc in range(NCH):
                        tp_ps = tpps_pool.tile([128, NP], bf16, tag="tp")
                        nc.tensor.transpose(
                            tp_ps[:], p_sb[:, jc * 128:(jc + 1) * 128], eye_sb[:]
                        )
                        pts = pt_pool.tile([128, NP], bf16, tag="pts")
                        nc.vector.tensor_copy(pts[:], tp_ps[:])
                        nc.tensor.matmul(
                            hist_ps[:],
                            pts[:],
                            u_sb[:, b, jc, :],
                            start=(b == 0 and jc == 0),
                            stop=(b == B - 1 and jc == NCH - 1),
                        )
                hist_sb = small_pool.tile([NP, NP], f32, tag="h0")
                nc.vector.tensor_copy(hist_sb[:], hist_ps[:])

            # ---- s-loads: ACT ring, throttled behind a-loads so the a
            # stream (critical path into the collective) gets HBM priority
            # early, then s saturates HBM during the collective window.
            for b in range(B):
                sld = nc.scalar.dma_start(
                    s_sb[:, b, :, :], s_d[b].rearrange("(c p) j -> p c j", p=128)
                )
                add_dep_helper(
                    sld.ins,
                    a_loads[min(b + 4, B - 1)].ins,
                    reason="throttle s-loads behind a-loads",
                )

            # ---- AllGather partial hists + local tree-sum + sigmoid ----
            cc_in = dram_pool.tile([NP, NP], f32, tag="ccin")
            cc_out = dram_pool.tile(
                [NCORES * NP, NP], f32, tag="ccout", addr_space="Shared"
            )
            nc.gpsimd.dma_start(cc_in[:], hist_sb[:])
            nc.gpsimd.collective_compute(
                "AllGather",
                mybir.AluOpType.bypass,
                replica_groups=[list(range(NCORES))],
                ins=[cc_in.opt()],
                outs=[cc_out.opt()],
            )
            hist8 = small_pool.tile([NP, NCORES, NP], f32, tag="h8")
            nc.gpsimd.dma_start(
                hist8[:], cc_out.opt().rearrange("(r p) q -> p r q", p=NP)
            )
            h4 = small_pool.tile([NP, 4, NP], f32, tag="h4")
            nc.vector.tensor_tensor(
                h4[:], hist8[:, 0:4, :], hist8[:, 4:8, :], mybir.AluOpType.add
            )
            h2 = small_pool.tile([NP, 2, NP], f32, tag="h2")
            nc.vector.tensor_tensor(
                h2[:], h4[:, 0:2, :], h4[:, 2:4, :], mybir.AluOpType.add
            )
            h1 = small_pool.tile([NP, NP], f32, tag="h1")
            nc.vector.tensor_tensor(
                h1[:], h2[:, 0, :], h2[:, 1, :], mybir.AluOpType.add
            )
            sc = small_pool.tile([NP, NP], bf16, tag="sc")
            nc.scalar.activation(
                sc[:], h1[:], mybir.ActivationFunctionType.Sigmoid
            )
            nc.vector.tensor_scalar_mul(sc[:], sc[:], ALPHA)

            # ---- Phase 2: out = s + GT.T @ VT, all bf16 ----
            with (
                tc.tile_pool(name="gtps", bufs=2, space="PSUM") as gtps_pool,
                tc.tile_pool(name="ops", bufs=3, space="PSUM") as ops_pool,
            ):
                for b in range(B):
                    gt_sb = gt_pool.tile([NP, SL], bf16, tag="gt")
                    for ib in range(NBLK):
                        gt_ps = gtps_pool.tile([NP, 512], f32, tag="gtp")
                        nc.tensor.matmul(
                            gt_ps[:],
                            sc[:],
                            vt_sb[:, b, ib * 512:(ib + 1) * 512],
                            start=True,
                            stop=True,
                        )
                        nc.vector.tensor_copy(
                            gt_sb[:, ib * 512:(ib + 1) * 512], gt_ps[:]
                        )
                    for h in range(2):  # half-batch store granularity (1 MB)
                        ot = o_pool.tile([128, 4, SL], bf16, tag="o")
                        for cc in range(4):
                            c = h * 4 + cc
                            o_ps = ops_pool.tile([128, SL], f32, tag="op")
                            for jb in range(NBLK):
                                sl_ = slice(jb * 512, (jb + 1) * 512)
                                nc.tensor.matmul(
                                    o_ps[:, sl_],
                                    gt_sb[:, c * 128:(c + 1) * 128],
                                    vt_sb[:, b, sl_],
                                    start=True,
                                    stop=False,
                                )
                                nc.tensor.matmul(
                                    o_ps[:, sl_],
                                    eye128_sb[:],
                                    s_sb[:, b, c, sl_],
                                    start=False,
                                    stop=True,
                                )
                            if c % 2 == 0:
                                nc.vector.tensor_copy(ot[:, cc, :], o_ps[:])
                            else:
                                nc.scalar.copy(ot[:, cc, :], o_ps[:])
                        nc.sync.dma_start(
                            out_d[b, h * 512:(h + 1) * 512, :].rearrange(
                                "(c p) j -> p c j", p=128
                            ),
                            ot[:],
                        )

    nc.compile()
    return nc


def _get_nc():
    if "nc" not in _CACHE:
        _CACHE["nc"] = _build_nc()
    return _CACHE["nc"]


def kernel(a_arc, s_arc, adds, pos, n_pos, _trace=False, _return_perf=False):
    from concourse.bass_utils import run_bass_kernel_spmd

    assert int(n_pos) == NP
    a = np.asarray(a_arc, dtype=np.float32)
    s = np.asarray(s_arc, dtype=np.float32)
    adds = np.asarray(adds)
    pos = np.asarray(pos)

    rng = np.arange(NP)
    eye = np.eye(NP, dtype=ml_dtypes.bfloat16)
    eye128 = np.eye(128, dtype=ml_dtypes.bfloat16)

    in_maps = []
    for k in range(NCORES):
        sl = slice(k * B, (k + 1) * B)
        adds_sh = adds[sl]
        pos_sh = pos[sl]
        # u[p, b, c, q] = [adds[b, c*128+p] == q]  (partition-major, fp8)
        u2 = (
            adds_sh.reshape(B, NCH, 128).transpose(2, 0, 1)[..., None] == rng
        ).astype(ml_dtypes.float8_e4m3)
        # vt[p, b, i] = [pos[b, i] == p]
        vt2 = (rng[:, None, None] == pos_sh[None, :, :]).astype(
            ml_dtypes.bfloat16
        )
        in_maps.append(
            {
                "a": np.ascontiguousarray(a[sl]).astype(ml_dtypes.bfloat16),
                "s": np.ascontiguousarray(s[sl]).astype(ml_dtypes.bfloat16),
                "u": np.ascontiguousarray(u2),
                "vt": np.ascontiguousarray(vt2),
                "eye": eye,
                "eye128": eye128,
            }
        )

    nc = _get_nc()
    res = run_bass_kernel_spmd(
        nc, in_maps, core_ids=list(range(NCORES)), trace=_trace
    )
    out = np.concatenate([r["out"] for r in res.results], axis=0).astype(np.float32)
    if _return_perf:
        return out, res
    return out


# revision 5
# speedup vs baseline: 1.1476x; 1.1476x over previous
"""Trainium2 Bass kernel for nn_EnsembleModel (histogram_binning).

Math:
  hist[p,q]  = sum_{b,i,j} [adds[b,i]==p] * a_arc[b,i,j] * [adds[b,j]==q]
  score      = sigmoid(hist)                                  # [50,50]
  out[b,i,j] = s_arc[b,i,j] + ALPHA * score[pos[b,i], pos[b,j]]

One-hot matmul formulation, all bf16 (one-hots exact; a/s rounded on host):

  phase 1 (per batch):  P[p,jblk] = sum_i U[i,p] A[i,j]  -- col-packed pairs:
                        jb0 -> PSUM rows 0-49, jb1 -> rows 64-113 (concurrent)
                        PT chunks = PE-transpose of P
                        hist     += PT.T @ U  -- col-packed pairs likewise
  AllGather(hist shards) -> local DVE tree-sum -> sigmoid * ALPHA
  phase 2 (per batch):  sc2 [50,128] holds score twice (cols 0-49, 64-113)
                        gt2 [128,SL] = sc2.T @ vt (both replicas in one MM)
                        out chunk pair (c, c+1): row-group-packed MMs
                          lhsT gt2[0:50, c]   rhs vt2[0:50]   -> o_ps_c
                          lhsT gt2[64:114,c+1] rhs vt2[64:114] -> o_ps_c1
                        add s: ACT copy PSUM->SBUF bf16 + DVE bf16 2x add
                        (some chunks: direct DVE PSUM-add for balance)

a-loads (SP ring) run at full HBM rate first; s-loads (ACT ring) are
throttled behind them so the collective latency (~40us mesh) hides under the
s stream.  A same-weight dummy-MM chain runs on the otherwise idle PE during
the collective to keep the HAM clock gate warm for phase 2.
"""

import numpy as np
import ml_dtypes

ALPHA = 0.3
NP = 50          # n_pos
SL = 1024        # sequence length
BZ = 64          # global batch
NCORES = 8
B = BZ // NCORES  # local batch per core
NCH = SL // 128   # 128-row chunks per matrix
NBLK = SL // 512  # 512-col blocks per matrix
WARM_MMS = 64     # dummy same-weight MMs bridging the collective window

_CACHE = {}


def _build_nc():
    import concourse.bacc as bacc
    import concourse.mybir as mybir
    import concourse.tile as tile
    from concourse.tile import add_dep_helper

    f32 = mybir.dt.float32
    bf16 = mybir.dt.bfloat16
    fp8 = mybir.dt.float8e4
    nc = bacc.Bacc(
        "TRN2", target_bir_lowering=False, debug=False, num_devices=NCORES
    )

    a_d = nc.dram_tensor("a", [B, SL, SL], bf16, kind="ExternalInput")
    s_d = nc.dram_tensor("s", [B, SL, SL], bf16, kind="ExternalInput")
    u_d = nc.dram_tensor("u", [128, B, NCH, NP], fp8, kind="ExternalInput")
    vt_d = nc.dram_tensor("vt", [128, B, SL], bf16, kind="ExternalInput")
    eye_d = nc.dram_tensor("eye", [NP, NP], bf16, kind="ExternalInput")
    eye128_d = nc.dram_tensor("eye128", [128, 128], bf16, kind="ExternalInput")
    out_d = nc.dram_tensor("out", [B, SL, SL], bf16, kind="ExternalOutput")

    with tile.TileContext(nc) as tc:
        with (
            tc.tile_pool(name="const", bufs=1) as const_pool,
            tc.tile_pool(name="apool", bufs=2) as a_pool,
            tc.tile_pool(name="opool", bufs=2) as o_pool,
            tc.tile_pool(name="gpool", bufs=3) as g_pool,
            tc.tile_pool(name="ppool", bufs=2) as p_pool,
            tc.tile_pool(name="ptsb", bufs=4) as pt_pool,
            tc.tile_pool(name="gtsb", bufs=2) as gt_pool,
            tc.tile_pool(name="small", bufs=1) as small_pool,
            tc.tile_pool(name="dram", bufs=1, space="DRAM") as dram_pool,
        ):
            u_sb = const_pool.tile([128, B, NCH, NP], fp8)
            eye_sb = const_pool.tile([NP, NP], bf16)
            eye128_sb = const_pool.tile([128, 128], bf16)
            vt_sb = const_pool.tile([128, B, SL], bf16)
            s_sb = const_pool.tile([128, B, NCH, SL], bf16)
            nc.sync.dma_start(eye_sb[:], eye_d[:])
            nc.sync.dma_start(eye128_sb[:], eye128_d[:])
            nc.sync.dma_start(u_sb[:], u_d[:])
            nc.sync.dma_start(vt_sb[:], vt_d[:])

            a_loads = []

            # ---- Phase 1: local histogram ----
            with (
                tc.tile_pool(name="histps", bufs=1, space="PSUM") as hist_pool,
                tc.tile_pool(name="pps", bufs=2, space="PSUM") as pps_pool,
                tc.tile_pool(name="tpps", bufs=2, space="PSUM") as tpps_pool,
            ):
                hist_ps = hist_pool.tile([128, NP], f32)
                for b in range(B):
                    at = a_pool.tile([128, NCH, SL], bf16, tag="a")
                    ld = nc.sync.dma_start(
                        at[:], a_d[b].rearrange("(c p) j -> p c j", p=128)
                    )
                    a_loads.append(ld)
                    # P: both 512-blocks concurrently (col groups 0 / 64),
                    # one PSUM bank, weights u_ic loaded once per ic.
                    p_ps = pps_pool.tile([128, 512], f32, tag="pp")
                    for ic in range(NCH):
                        st_, sp_ = (ic == 0), (ic == NCH - 1)
                        nc.tensor.matmul(
                            p_ps[0:NP, :],
                            u_sb[:, b, ic, :],
                            at[:, ic, 0:512],
                            start=st_, stop=sp_,
                        )
                        nc.tensor.matmul(
                            p_ps[64:64 + NP, :],
                            u_sb[:, b, ic, :],
                            at[:, ic, 512:1024],
                            start=st_, stop=sp_,
                            tile_position=(0, 64),
                        )
                    p_sb = p_pool.tile([NP, SL], bf16, tag="p")
                    nc.scalar.copy(p_sb[:, 0:512], p_ps[0:NP, :])
                    nc.scalar.copy(p_sb[:, 512:1024], p_ps[64:64 + NP, :])
                    # hist += PT.T @ U, col-packed in jc pairs.
                    for jc in range(NCH):
                        tp_ps = tpps_pool.tile([128, NP], bf16, tag="tp")
                        nc.tensor.transpose(
                            tp_ps[:], p_sb[:, jc * 128:(jc + 1) * 128], eye_sb[:]
                        )
                        pts = pt_pool.tile([128, NP], bf16, tag="pts")
                        nc.vector.tensor_copy(pts[:], tp_ps[:])
                        first = (b == 0 and jc < 2)
                        last = (b == B - 1 and jc >= NCH - 2)
                        if jc % 2 == 0:
                            nc.tensor.matmul(
                                hist_ps[0:NP, :], pts[:], u_sb[:, b, jc, :],
                                start=first, stop=last,
                            )
                        else:
                            nc.tensor.matmul(
                                hist_ps[64:64 + NP, :], pts[:],
                                u_sb[:, b, jc, :],
                                start=first, stop=last,
                                tile_position=(0, 64),
                            )
                htmp = small_pool.tile([NP, NP], f32, tag="ht")
                nc.vector.tensor_copy(htmp[:], hist_ps[64:64 + NP, :])
                hist_sb = small_pool.tile([NP, NP], f32, tag="h0")
                nc.vector.tensor_tensor(
                    hist_sb[:], hist_ps[0:NP, :], htmp[:], mybir.AluOpType.add
                )

            # ---- s-loads: ACT ring, gated so a-loads keep full BW early
            # and s streams through the collective window.
            for b in range(B):
                sld = nc.scalar.dma_start(
                    s_sb[:, b, :, :], s_d[b].rearrange("(c p) j -> p c j", p=128)
                )
                add_dep_helper(
                    sld.ins,
                    a_loads[min(b + 3, B - 1)].ins,
                    reason="throttle s-loads behind a-loads",
                )

            # ---- AllGather partial hists + local sum + sigmoid ----
            cc_in = dram_pool.tile([NP, NP], f32, tag="ccin")
            cc_out = dram_pool.tile(
                [NCORES * NP, NP], f32, tag="ccout", addr_space="Shared"
            )
            nc.gpsimd.dma_start(cc_in[:], hist_sb[:])
            nc.gpsimd.collective_compute(
                "AllGather",
                mybir.AluOpType.bypass,
                replica_groups=[list(range(NCORES))],
                ins=[cc_in.opt()],
                outs=[cc_out.opt()],
            )
            hist8 = small_pool.tile([NP, NCORES, NP], f32, tag="h8")
            nc.gpsimd.dma_start(
                hist8[:], cc_out.opt().rearrange("(r p) q -> p r q", p=NP)
            )
            h4 = small_pool.tile([NP, 4, NP], f32, tag="h4")
            nc.vector.tensor_tensor(
                h4[:], hist8[:, 0:4, :], hist8[:, 4:8, :], mybir.AluOpType.add
            )
            h2 = small_pool.tile([NP, 2, NP], f32, tag="h2")
            nc.vector.tensor_tensor(
                h2[:], h4[:, 0:2, :], h4[:, 2:4, :], mybir.AluOpType.add
            )
            h1 = small_pool.tile([NP, NP], f32, tag="h1")
            nc.vector.tensor_tensor(
                h1[:], h2[:, 0, :], h2[:, 1, :], mybir.AluOpType.add
            )
            # score duplicated at cols 0-49 and 64-113 so one gt-MM emits
            # both row-group replicas of gt.
            sc2 = small_pool.tile([NP, 128], bf16, tag="sc2")
            nc.vector.memset(sc2[:], 0.0)
            nc.scalar.activation(
                sc2[:, 0:NP], h1[:], mybir.ActivationFunctionType.Sigmoid
            )
            nc.scalar.activation(
                sc2[:, 64:64 + NP], h1[:], mybir.ActivationFunctionType.Sigmoid
            )
            nc.vector.tensor_scalar_mul(sc2[:], sc2[:], ALPHA)

            # ---- Phase 2 ----
            with (
                tc.tile_pool(name="gtps", bufs=1, space="PSUM") as gtps_pool,
                tc.tile_pool(name="warmps", bufs=1, space="PSUM") as warm_pool,
                tc.tile_pool(name="ops", bufs=3, space="PSUM") as ops_pool,
            ):
                # Same-weight dummy chain: keeps PE activity up through the
                # collective window so phase 2 starts at the warm clock.
                if WARM_MMS:
                    wps = warm_pool.tile([128, 512], f32, tag="warm")
                    for _ in range(WARM_MMS):
                        nc.tensor.matmul(
                            wps[:], eye128_sb[:], s_sb[:, 0, 0, 0:512],
                            start=True, stop=True,
                        )
                for b in range(B):
                    gt2 = gt_pool.tile([128, SL], bf16, tag="gt")
                    for ib in range(NBLK):
                        gt_ps = gtps_pool.tile([128, 512], f32, tag="gtp")
                        nc.tensor.matmul(
                            gt_ps[:],
                            sc2[:],
                            vt_sb[0:NP, b, ib * 512:(ib + 1) * 512],
                            start=True, stop=True,
                        )
                        eng = nc.vector if ib == 0 else nc.scalar
                        eng_copy = (
                            nc.vector.tensor_copy if ib == 0 else nc.scalar.copy
                        )
                        eng_copy(gt2[:, ib * 512:(ib + 1) * 512], gt_ps[:])
                    for h in range(4):  # quarter-batch stores (512 KB)
                        ot = o_pool.tile([128, 2, SL], bf16, tag="o")
                        for cc in range(2):
                            c = h * 2 + cc
                            o_ps = ops_pool.tile([128, SL], f32, tag="op")
                            lo = (c % 2) * 64  # row-group 0-49 or 64-113
                            tp = None if c % 2 == 0 else (64, 0)
                            for jb in range(NBLK):
                                sl_ = slice(jb * 512, (jb + 1) * 512)
                                kw = {} if tp is None else {"tile_position": tp}
                                nc.tensor.matmul(
                                    o_ps[:, sl_],
                                    gt2[lo:lo + NP, c * 128:(c + 1) * 128],
                                    vt_sb[lo:lo + NP, b, sl_],
                                    start=True, stop=True,
                                    **kw,
                                )
                            if c % 8 == 7:
                                # direct PSUM add on DVE (balances ACT)
                                nc.vector.tensor_tensor(
                                    ot[:, cc, :], s_sb[:, b, c, :], o_ps[:],
                                    mybir.AluOpType.add,
                                )
                            else:
                                gsb = g_pool.tile([128, SL], bf16, tag="g")
                                nc.scalar.copy(gsb[:], o_ps[:])
                                nc.vector.tensor_tensor(
                                    ot[:, cc, :], s_sb[:, b, c, :], gsb[:],
                                    mybir.AluOpType.add,
                                )
                        nc.sync.dma_start(
                            out_d[b, h * 256:(h + 1) * 256, :].rearrange(
                                "(c p) j -> p c j", p=128
                            ),
                            ot[:],
                        )

    nc.compile()
    return nc


def _get_nc():
    if "nc" not in _CACHE:
        _CACHE["nc"] = _build_nc()
    return _CACHE["nc"]


def kernel(a_arc, s_arc, adds, pos, n_pos, _trace=False, _return_perf=False):
    from concourse.bass_utils import run_bass_kernel_spmd

    assert int(n_pos) == NP
    a = np.asarray(a_arc, dtype=np.float32)
    s = np.asarray(s_arc, dtype=np.float32)
    adds = np.asarray(adds)
    pos = np.asarray(pos)

    rng = np.arange(NP)
    eye = np.eye(NP, dtype=ml_dtypes.bfloat16)
    eye128 = np.eye(128, dtype=ml_dtypes.bfloat16)

    in_maps = []
    for k in range(NCORES):
        sl = slice(k * B, (k + 1) * B)
        adds_sh = adds[sl]
        pos_sh = pos[sl]
        # u[p, b, c, q] = [adds[b, c*128+p] == q]  (partition-major, fp8)
        u2 = (
            adds_sh.reshape(B, NCH, 128).transpose(2, 0, 1)[..., None] == rng
        ).astype(ml_dtypes.float8_e4m3)
        # vt2[p, b, i]: one-hot [pos==p] replicated at rows 0-49 and 64-113
        # for row-group-packed matmul pairs.
        oh = (rng[:, None, None] == pos_sh[None, :, :]).astype(
            ml_dtypes.bfloat16
        )
        vt2 = np.zeros((128, B, SL), dtype=ml_dtypes.bfloat16)
        vt2[0:NP] = oh
        vt2[64:64 + NP] = oh
        in_maps.append(
            {
                "a": np.ascontiguousarray(a[sl]).astype(ml_dtypes.bfloat16),
                "s": np.ascontiguousarray(s[sl]).astype(ml_dtypes.bfloat16),
                "u": np.ascontiguousarray(u2),
                "vt": np.ascontiguousarray(vt2),
                "eye": eye,
                "eye128": eye128,
            }
        )

    nc = _get_nc()
    res = run_bass_kernel_spmd(
        nc, in_maps, core_ids=list(range(NCORES)), trace=_trace
    )
    out = np.concatenate([r["out"] for r in res.results], axis=0).astype(np.float32)
    if _return_perf:
        return out, res
    return out


# revision 7
# speedup vs baseline: 1.2774x; 1.1130x over previous
"""Trainium2 Bass kernel for nn_EnsembleModel (histogram_binning).

Math:
  hist[p,q]  = sum_{b,i,j} [adds[b,i]==p] * a_arc[b,i,j] * [adds[b,j]==q]
  score      = sigmoid(hist)                                  # [50,50]
  out[b,i,j] = s_arc[b,i,j] + ALPHA * score[pos[b,i], pos[b,j]]

One-hot matmul formulation, all bf16 (one-hots exact; a/s rounded on host):

  phase 1 (per batch):  P[p,jblk] = sum_i U[i,p] A[i,j]  -- col-packed pairs
                        (jb0 -> PSUM rows 0-49, jb1 -> rows 64-113, concurrent)
                        PT chunks = PE-transpose of P (pipelined one batch
                        behind the P matmuls so ACT copies never stall PE)
                        hist     += PT.T @ U  -- col-packed jc pairs
  AllGather(hist shards) -> local DVE tree-sum -> sigmoid * ALPHA
  phase 2 (per batch):  sc2 [50,128] holds score twice (cols 0-49, 64-113)
                        gt2 [128,SL] = sc2.T @ vt (both replicas in one MM)
                        chunk pairs (c even, c+1): 4 MMs interleaved so the
                        row-group-0 and row-group-64 MMs run concurrently
                        s-add split 3 ways to balance engines:
                          direct DVE PSUM-add | ACT copy + DVE bf16 add |
                          ACT copy + GpSimd bf16 add

DMA schedule: u first, then a at full HBM rate (SP ring, 1MB half-batch
loads); vt2 and the s stream (ACT ring) are dependency-throttled behind the
a stream so the ~40us mesh-AllGather latency hides under the s-loads.
"""

import numpy as np
import ml_dtypes

ALPHA = 0.3
NP = 50          # n_pos
SL = 1024        # sequence length
BZ = 64          # global batch
NCORES = 8
B = BZ // NCORES  # local batch per core
NCH = SL // 128   # 128-row chunks per matrix
NBLK = SL // 512  # 512-col blocks per matrix

_CACHE = {}


def _build_nc():
    import concourse.bacc as bacc
    import concourse.mybir as mybir
    import concourse.tile as tile
    from concourse.tile import add_dep_helper

    f32 = mybir.dt.float32
    bf16 = mybir.dt.bfloat16
    fp8 = mybir.dt.float8e4
    nc = bacc.Bacc(
        "TRN2", target_bir_lowering=False, debug=False, num_devices=NCORES
    )

    a_d = nc.dram_tensor("a", [B, SL, SL], bf16, kind="ExternalInput")
    s_d = nc.dram_tensor("s", [B, SL, SL], bf16, kind="ExternalInput")
    u_d = nc.dram_tensor("u", [128, B, NCH, NP], fp8, kind="ExternalInput")
    vt_d = nc.dram_tensor("vt", [128, B, SL], bf16, kind="ExternalInput")
    eye_d = nc.dram_tensor("eye", [NP, NP], bf16, kind="ExternalInput")
    out_d = nc.dram_tensor("out", [B, SL, SL], bf16, kind="ExternalOutput")

    with tile.TileContext(nc) as tc:
        with (
            tc.tile_pool(name="const", bufs=1) as const_pool,
            tc.tile_pool(name="apool", bufs=4) as a_pool,
            tc.tile_pool(name="opool", bufs=2) as o_pool,
            tc.tile_pool(name="gpool", bufs=3) as g_pool,
            tc.tile_pool(name="ppool", bufs=2) as p_pool,
            tc.tile_pool(name="ptsb", bufs=8) as pt_pool,
            tc.tile_pool(name="gtsb", bufs=2) as gt_pool,
            tc.tile_pool(name="small", bufs=1) as small_pool,
            tc.tile_pool(name="dram", bufs=1, space="DRAM") as dram_pool,
        ):
            u_sb = const_pool.tile([128, B, NCH, NP], fp8)
            eye_sb = const_pool.tile([NP, NP], bf16)
            vt_sb = const_pool.tile([128, B, SL], bf16)
            s_sb = const_pool.tile([128, B, NCH, SL], bf16)
            nc.sync.dma_start(eye_sb[:], eye_d[:])
            nc.sync.dma_start(u_sb[:], u_d[:])

            a_loads = []

            # ---- Phase 1: local histogram ----
            with (
                tc.tile_pool(name="histps", bufs=1, space="PSUM") as hist_pool,
                tc.tile_pool(name="pps", bufs=2, space="PSUM") as pps_pool,
                tc.tile_pool(name="tpps", bufs=2, space="PSUM") as tpps_pool,
            ):
                hist_ps = hist_pool.tile([128, NP], f32)
                p_hist = []  # (b, p_sb) pending transpose+hist work

                def emit_hist(b, p_sb):
                    # 8 transposes first, then col-packed hist MM pairs.
                    pts_l = []
                    for jc in range(NCH):
                        tp_ps = tpps_pool.tile([128, NP], bf16, tag="tp")
                        nc.tensor.transpose(
                            tp_ps[:], p_sb[:, jc * 128:(jc + 1) * 128],
                            eye_sb[:],
                        )
                        pts = pt_pool.tile([128, NP], bf16, tag="pts")
                        nc.vector.tensor_copy(pts[:], tp_ps[:])
                        pts_l.append(pts)
                    first = b == 0
                    last = b == B - 1
                    for jc in range(0, NCH, 2):
                        nc.tensor.matmul(
                            hist_ps[0:NP, :], pts_l[jc][:], u_sb[:, b, jc, :],
                            start=(first and jc == 0),
                            stop=(last and jc == NCH - 2),
                        )
                        nc.tensor.matmul(
                            hist_ps[64:64 + NP, :], pts_l[jc + 1][:],
                            u_sb[:, b, jc + 1, :],
                            start=(first and jc == 0),
                            stop=(last and jc == NCH - 2),
                            tile_position=(0, 64),
                        )

                for b in range(B):
                    at_lo = a_pool.tile([128, 4, SL], bf16, tag="a")
                    at_hi = a_pool.tile([128, 4, SL], bf16, tag="a")
                    a_loads.append(nc.sync.dma_start(
                        at_lo[:],
                        a_d[b, 0:512, :].rearrange("(c p) j -> p c j", p=128),
                    ))
                    a_loads.append(nc.sync.dma_start(
                        at_hi[:],
                        a_d[b, 512:1024, :].rearrange("(c p) j -> p c j", p=128),
                    ))
                    p_ps = pps_pool.tile([128, 512], f32, tag="pp")
                    for ic in range(NCH):
                        at = at_lo if ic < 4 else at_hi
                        icc = ic % 4
                        st_, sp_ = (ic == 0), (ic == NCH - 1)
                        nc.tensor.matmul(
                            p_ps[0:NP, :],
                            u_sb[:, b, ic, :],
                            at[:, icc, 0:512],
                            start=st_, stop=sp_,
                        )
                        nc.tensor.matmul(
                            p_ps[64:64 + NP, :],
                            u_sb[:, b, ic, :],
                            at[:, icc, 512:1024],
                            start=st_, stop=sp_,
                            tile_position=(0, 64),
                        )
                    p_sb = p_pool.tile([NP, SL], bf16, tag="p")
                    nc.scalar.copy(p_sb[:, 0:512], p_ps[0:NP, :])
                    nc.scalar.copy(p_sb[:, 512:1024], p_ps[64:64 + NP, :])
                    p_hist.append((b, p_sb))
                    # transpose+hist for the PREVIOUS batch: PE never waits
                    # on this batch's ACT copies.
                    if len(p_hist) > 1:
                        emit_hist(*p_hist.pop(0))
                emit_hist(*p_hist.pop(0))

                htmp = small_pool.tile([NP, NP], f32, tag="ht")
                nc.vector.tensor_copy(htmp[:], hist_ps[64:64 + NP, :])
                hist_sb = small_pool.tile([NP, NP], f32, tag="h0")
                nc.vector.tensor_tensor(
                    hist_sb[:], hist_ps[0:NP, :], htmp[:], mybir.AluOpType.add
                )

            # ---- vt2 + s loads: ACT ring, throttled so a keeps full BW
            # and s streams through the collective window.
            vld = nc.scalar.dma_start(vt_sb[:], vt_d[:])
            add_dep_helper(
                vld.ins, a_loads[9].ins, reason="vt2 after a mostly done"
            )
            for b in range(B):
                sld = nc.scalar.dma_start(
                    s_sb[:, b, :, :], s_d[b].rearrange("(c p) j -> p c j", p=128)
                )
                add_dep_helper(
                    sld.ins,
                    a_loads[14].ins,
                    reason="s-loads after a stream",
                )

            # ---- AllGather partial hists + local sum + sigmoid ----
            cc_in = dram_pool.tile([NP, NP], f32, tag="ccin")
            cc_out = dram_pool.tile(
                [NCORES * NP, NP], f32, tag="ccout", addr_space="Shared"
            )
            nc.gpsimd.dma_start(cc_in[:], hist_sb[:])
            nc.gpsimd.collective_compute(
                "AllGather",
                mybir.AluOpType.bypass,
                replica_groups=[list(range(NCORES))],
                ins=[cc_in.opt()],
                outs=[cc_out.opt()],
            )
            hist8 = small_pool.tile([NP, NCORES, NP], f32, tag="h8")
            nc.gpsimd.dma_start(
                hist8[:], cc_out.opt().rearrange("(r p) q -> p r q", p=NP)
            )
            h4 = small_pool.tile([NP, 4, NP], f32, tag="h4")
            nc.vector.tensor_tensor(
                h4[:], hist8[:, 0:4, :], hist8[:, 4:8, :], mybir.AluOpType.add
            )
            h2 = small_pool.tile([NP, 2, NP], f32, tag="h2")
            nc.vector.tensor_tensor(
                h2[:], h4[:, 0:2, :], h4[:, 2:4, :], mybir.AluOpType.add
            )
            h1 = small_pool.tile([NP, NP], f32, tag="h1")
            nc.vector.tensor_tensor(
                h1[:], h2[:, 0, :], h2[:, 1, :], mybir.AluOpType.add
            )
            sc2 = small_pool.tile([NP, 128], bf16, tag="sc2")
            nc.vector.memset(sc2[:], 0.0)
            nc.scalar.activation(
                sc2[:, 0:NP], h1[:], mybir.ActivationFunctionType.Sigmoid
            )
            nc.scalar.activation(
                sc2[:, 64:64 + NP], h1[:], mybir.ActivationFunctionType.Sigmoid
            )
            nc.vector.tensor_scalar_mul(sc2[:], sc2[:], ALPHA)

            # ---- Phase 2 ----
            with (
                tc.tile_pool(name="gtps", bufs=1, space="PSUM") as gtps_pool,
                tc.tile_pool(name="ops", bufs=3, space="PSUM") as ops_pool,
            ):
                for b in range(B):
                    gt2 = gt_pool.tile([128, SL], bf16, tag="gt")
                    for ib in range(NBLK):
                        gt_ps = gtps_pool.tile([128, 512], f32, tag="gtp")
                        nc.tensor.matmul(
                            gt_ps[:],
                            sc2[:],
                            vt_sb[0:NP, b, ib * 512:(ib + 1) * 512],
                            start=True, stop=True,
                        )
                        if ib == 0:
                            nc.vector.tensor_copy(
                                gt2[:, ib * 512:(ib + 1) * 512], gt_ps[:]
                            )
                        else:
                            nc.scalar.copy(
                                gt2[:, ib * 512:(ib + 1) * 512], gt_ps[:]
                            )
                    for h in range(4):  # chunk pair per h; 512 KB stores
                        ot = o_pool.tile([128, 2, SL], bf16, tag="o")
                        c0, c1 = 2 * h, 2 * h + 1
                        ps0 = ops_pool.tile([128, SL], f32, tag="op")
                        ps1 = ops_pool.tile([128, SL], f32, tag="op")
                        # interleave so row-group-0 / row-group-64 MMs pair up
                        for jb in range(NBLK):
                            sl_ = slice(jb * 512, (jb + 1) * 512)
                            nc.tensor.matmul(
                                ps0[:, sl_],
                                gt2[0:NP, c0 * 128:(c0 + 1) * 128],
                                vt_sb[0:NP, b, sl_],
                                start=True, stop=True,
                            )
                            nc.tensor.matmul(
                                ps1[:, sl_],
                                gt2[64:64 + NP, c1 * 128:(c1 + 1) * 128],
                                vt_sb[64:64 + NP, b, sl_],
                                start=True, stop=True,
                                tile_position=(64, 0),
                            )
                        for cc, o_ps in ((0, ps0), (1, ps1)):
                            c = 2 * h + cc
                            k = b * NCH + c
                            m = k % 16
                            if m < 6:
                                # direct PSUM add on DVE
                                nc.vector.tensor_tensor(
                                    ot[:, cc, :], s_sb[:, b, c, :], o_ps[:],
                                    mybir.AluOpType.add,
                                )
                            else:
                                gsb = g_pool.tile([128, SL], bf16, tag="g")
                                nc.scalar.copy(gsb[:], o_ps[:])
                                if m < 13:
                                    nc.vector.tensor_tensor(
                                        ot[:, cc, :], s_sb[:, b, c, :],
                                        gsb[:], mybir.AluOpType.add,
                                    )
                                else:
                                    nc.gpsimd.tensor_tensor(
                                        ot[:, cc, :], s_sb[:, b, c, :],
                                        gsb[:], mybir.AluOpType.add,
                                    )
                        nc.sync.dma_start(
                            out_d[b, h * 256:(h + 1) * 256, :].rearrange(
                                "(c p) j -> p c j", p=128
                            ),
                            ot[:],
                        )

    nc.compile()
    return nc


def _get_nc():
    if "nc" not in _CACHE:
        _CACHE["nc"] = _build_nc()
    return _CACHE["nc"]


def kernel(a_arc, s_arc, adds, pos, n_pos, _trace=False, _return_perf=False):
    from concourse.bass_utils import run_bass_kernel_spmd

    assert int(n_pos) == NP
    a = np.asarray(a_arc, dtype=np.float32)
    s = np.asarray(s_arc, dtype=np.float32)
    adds = np.asarray(adds)
    pos = np.asarray(pos)

    rng = np.arange(NP)
    eye = np.eye(NP, dtype=ml_dtypes.bfloat16)

    in_maps = []
    for k in range(NCORES):
        sl = slice(k * B, (k + 1) * B)
        adds_sh = adds[sl]
        pos_sh = pos[sl]
        # u[p, b, c, q] = [adds[b, c*128+p] == q]  (partition-major, fp8)
        u2 = (
            adds_sh.reshape(B, NCH, 128).transpose(2, 0, 1)[..., None] == rng
        ).astype(ml_dtypes.float8_e4m3)
        # vt2[p, b, i]: one-hot [pos==p] replicated at rows 0-49 and 64-113
        oh = (rng[:, None, None] == pos_sh[None, :, :]).astype(
            ml_dtypes.bfloat16
        )
        vt2 = np.zeros((128, B, SL), dtype=ml_dtypes.bfloat16)
        vt2[0:NP] = oh
        vt2[64:64 + NP] = oh
        in_maps.append(
            {
                "a": np.ascontiguousarray(a[sl]).astype(ml_dtypes.bfloat16),
                "s": np.ascontiguousarray(s[sl]).astype(ml_dtypes.bfloat16),
                "u": np.ascontiguousarray(u2),
                "vt": np.ascontiguousarray(vt2),
                "eye": eye,
            }
        )

    nc = _get_nc()
    res = run_bass_kernel_spmd(
        nc, in_maps, core_ids=list(range(NCORES)), trace=_trace
    )
    out = np.concatenate([r["out"] for r in res.results], axis=0).astype(np.float32)
    if _return_perf:
        return out, res
    return out


# revision 9
# speedup vs baseline: 1.4764x; 1.1558x over previous
"""Trainium2 Bass kernel for nn_EnsembleModel (histogram_binning).

Math:
  hist[p,q]  = sum_{b,i,j} [adds[b,i]==p] * a_arc[b,i,j] * [adds[b,j]==q]
  score      = sigmoid(hist)                                  # [50,50]
  out[b,i,j] = s_arc[b,i,j] + ALPHA * score[pos[b,i], pos[b,j]]

One-hot matmul formulation, all bf16 (one-hots exact; a/s rounded on host):

  phase 1 (per batch):  P[p,jblk] = sum_i U[i,p] A[i,j]  -- col-packed pairs
                        (jb0 -> PSUM rows 0-49, jb1 -> rows 64-113, concurrent)
                        PT chunks = PE-transpose of P (pipelined one batch
                        behind the P matmuls so ACT copies never stall PE)
                        hist     += PT.T @ U  -- col-packed jc pairs
  AllGather(hist shards) -> local DVE tree-sum -> sigmoid * ALPHA
  phase 2 (per batch):  sc2 [50,128] holds score twice (cols 0-49, 64-113)
                        gt2 [128,SL] = sc2.T @ vt (both replicas in one MM)
                        chunk pairs (c even, c+1): 4 MMs interleaved so the
                        row-group-0 and row-group-64 MMs run concurrently
                        s-add split 3 ways to balance engines:
                          direct DVE PSUM-add | ACT copy + DVE bf16 add |
                          ACT copy + GpSimd bf16 add

DMA schedule: u first, then a at full HBM rate (SP ring, 1MB half-batch
loads); vt2 and the s stream (ACT ring) are dependency-throttled behind the
a stream so the ~40us mesh-AllGather latency hides under the s-loads.
"""

import numpy as np
import ml_dtypes

ALPHA = 0.3
NP = 50          # n_pos
SL = 1024        # sequence length
BZ = 64          # global batch
NCORES = 8
B = BZ // NCORES  # local batch per core
NCH = SL // 128   # 128-row chunks per matrix
NBLK = SL // 512  # 512-col blocks per matrix

_CACHE = {}


def _build_nc():
    import concourse.bacc as bacc
    import concourse.mybir as mybir
    import concourse.tile as tile
    from concourse.tile import add_dep_helper

    f32 = mybir.dt.float32
    bf16 = mybir.dt.bfloat16
    fp8 = mybir.dt.float8e4
    nc = bacc.Bacc(
        "TRN2", target_bir_lowering=False, debug=False, num_devices=NCORES
    )

    a_d = nc.dram_tensor("a", [B, SL, SL], bf16, kind="ExternalInput")
    s_d = nc.dram_tensor("s", [B, SL, SL], bf16, kind="ExternalInput")
    u_d = nc.dram_tensor("u", [128, B, NCH, NP], fp8, kind="ExternalInput")
    vt_d = nc.dram_tensor("vt", [128, B, SL], bf16, kind="ExternalInput")
    eye_d = nc.dram_tensor("eye", [NP, NP], bf16, kind="ExternalInput")
    out_d = nc.dram_tensor("out", [B, SL, SL], bf16, kind="ExternalOutput")

    with tile.TileContext(nc) as tc:
        with (
            tc.tile_pool(name="const", bufs=1) as const_pool,
            tc.tile_pool(name="apool", bufs=3) as a_pool,
            tc.tile_pool(name="opool", bufs=6) as o_pool,
            tc.tile_pool(name="gpool", bufs=3) as g_pool,
            tc.tile_pool(name="ppool", bufs=2) as p_pool,
            tc.tile_pool(name="ptsb", bufs=8) as pt_pool,
            tc.tile_pool(name="gtsb", bufs=2) as gt_pool,
            tc.tile_pool(name="small", bufs=1) as small_pool,
            tc.tile_pool(name="dram", bufs=1, space="DRAM") as dram_pool,
        ):
            u_sb = const_pool.tile([128, B, NCH, NP], fp8)
            eye_sb = const_pool.tile([NP, NP], bf16)
            vt_sb = const_pool.tile([128, B, SL], bf16)
            s_sb = const_pool.tile([128, B, NCH, SL], bf16)
            nc.sync.dma_start(eye_sb[:], eye_d[:])
            nc.sync.dma_start(u_sb[:], u_d[:])

            a_loads = []

            # ---- Phase 1: local histogram ----
            with (
                tc.tile_pool(name="histps", bufs=1, space="PSUM") as hist_pool,
                tc.tile_pool(name="pps", bufs=2, space="PSUM") as pps_pool,
                tc.tile_pool(name="tpps", bufs=2, space="PSUM") as tpps_pool,
            ):
                hist_ps = hist_pool.tile([128, NP], f32)
                p_hist = []  # (b, p_sb) pending transpose+hist work

                def emit_hist(b, p_sb):
                    # 8 transposes first, then col-packed hist MM pairs.
                    pts_l = []
                    for jc in range(NCH):
                        tp_ps = tpps_pool.tile([128, NP], bf16, tag="tp")
                        nc.tensor.transpose(
                            tp_ps[:], p_sb[:, jc * 128:(jc + 1) * 128],
                            eye_sb[:],
                        )
                        pts = pt_pool.tile([128, NP], bf16, tag="pts")
                        nc.vector.tensor_copy(pts[:], tp_ps[:])
                        pts_l.append(pts)
                    first = b == 0
                    last = b == B - 1
                    for jc in range(0, NCH, 2):
                        nc.tensor.matmul(
                            hist_ps[0:NP, :], pts_l[jc][:], u_sb[:, b, jc, :],
                            start=(first and jc == 0),
                            stop=(last and jc == NCH - 2),
                        )
                        nc.tensor.matmul(
                            hist_ps[64:64 + NP, :], pts_l[jc + 1][:],
                            u_sb[:, b, jc + 1, :],
                            start=(first and jc == 0),
                            stop=(last and jc == NCH - 2),
                            tile_position=(0, 64),
                        )

                for b in range(B):
                    # transpose+hist for the PREVIOUS batch first: these fill
                    # the PE while this batch's a-tiles are still loading.
                    if p_hist:
                        emit_hist(*p_hist.pop(0))
                    at_lo = a_pool.tile([128, 4, SL], bf16, tag="a")
                    at_hi = a_pool.tile([128, 4, SL], bf16, tag="a")
                    a_loads.append(nc.sync.dma_start(
                        at_lo[:],
                        a_d[b, 0:512, :].rearrange("(c p) j -> p c j", p=128),
                    ))
                    a_loads.append(nc.sync.dma_start(
                        at_hi[:],
                        a_d[b, 512:1024, :].rearrange("(c p) j -> p c j", p=128),
                    ))
                    p_ps = pps_pool.tile([128, 512], f32, tag="pp")
                    for ic in range(NCH):
                        at = at_lo if ic < 4 else at_hi
                        icc = ic % 4
                        st_, sp_ = (ic == 0), (ic == NCH - 1)
                        nc.tensor.matmul(
                            p_ps[0:NP, :],
                            u_sb[:, b, ic, :],
                            at[:, icc, 0:512],
                            start=st_, stop=sp_,
                        )
                        nc.tensor.matmul(
                            p_ps[64:64 + NP, :],
                            u_sb[:, b, ic, :],
                            at[:, icc, 512:1024],
                            start=st_, stop=sp_,
                            tile_position=(0, 64),
                        )
                    p_sb = p_pool.tile([NP, SL], bf16, tag="p")
                    nc.scalar.copy(p_sb[:, 0:512], p_ps[0:NP, :])
                    nc.scalar.copy(p_sb[:, 512:1024], p_ps[64:64 + NP, :])
                    p_hist.append((b, p_sb))
                emit_hist(*p_hist.pop(0))

                htmp = small_pool.tile([NP, NP], f32, tag="ht")
                nc.vector.tensor_copy(htmp[:], hist_ps[64:64 + NP, :])
                hist_sb = small_pool.tile([NP, NP], f32, tag="h0")
                nc.vector.tensor_tensor(
                    hist_sb[:], hist_ps[0:NP, :], htmp[:], mybir.AluOpType.add
                )

            # ---- vt2 + s loads: ACT ring, throttled so a keeps full BW
            # and s streams through the collective window.
            vld = nc.scalar.dma_start(vt_sb[:], vt_d[:])
            add_dep_helper(
                vld.ins, a_loads[9].ins, reason="vt2 after a mostly done"
            )
            for b in range(B):
                sld = nc.scalar.dma_start(
                    s_sb[:, b, :, :], s_d[b].rearrange("(c p) j -> p c j", p=128)
                )
                add_dep_helper(
                    sld.ins,
                    a_loads[14].ins,
                    reason="s-loads after a stream",
                )

            # ---- Tiny dummy AllGather issued first: absorbs any
            # first-collective setup cost while phase 1 still runs.
            ccw_in = dram_pool.tile([2, 2], f32, tag="ccwin")
            ccw_out = dram_pool.tile(
                [2 * NCORES, 2], f32, tag="ccwout", addr_space="Shared"
            )
            nc.gpsimd.collective_compute(
                "AllGather",
                mybir.AluOpType.bypass,
                replica_groups=[list(range(NCORES))],
                ins=[ccw_in.opt()],
                outs=[ccw_out.opt()],
            )

            # ---- AllGather partial hists + local sum + sigmoid ----
            cc_in = dram_pool.tile([NP, NP], f32, tag="ccin")
            cc_out = dram_pool.tile(
                [NCORES * NP, NP], f32, tag="ccout", addr_space="Shared"
            )
            nc.gpsimd.dma_start(cc_in[:], hist_sb[:])
            nc.gpsimd.collective_compute(
                "AllGather",
                mybir.AluOpType.bypass,
                replica_groups=[list(range(NCORES))],
                ins=[cc_in.opt()],
                outs=[cc_out.opt()],
            )
            hist8 = small_pool.tile([NP, NCORES, NP], f32, tag="h8")
            nc.gpsimd.dma_start(
                hist8[:], cc_out.opt().rearrange("(r p) q -> p r q", p=NP)
            )
            h4 = small_pool.tile([NP, 4, NP], f32, tag="h4")
            nc.vector.tensor_tensor(
                h4[:], hist8[:, 0:4, :], hist8[:, 4:8, :], mybir.AluOpType.add
            )
            h2 = small_pool.tile([NP, 2, NP], f32, tag="h2")
            nc.vector.tensor_tensor(
                h2[:], h4[:, 0:2, :], h4[:, 2:4, :], mybir.AluOpType.add
            )
            h1 = small_pool.tile([NP, NP], f32, tag="h1")
            nc.vector.tensor_tensor(
                h1[:], h2[:, 0, :], h2[:, 1, :], mybir.AluOpType.add
            )
            sc2 = small_pool.tile([NP, 128], bf16, tag="sc2")
            nc.vector.memset(sc2[:], 0.0)
            nc.scalar.activation(
                sc2[:, 0:NP], h1[:], mybir.ActivationFunctionType.Sigmoid
            )
            nc.scalar.activation(
                sc2[:, 64:64 + NP], h1[:], mybir.ActivationFunctionType.Sigmoid
            )
            nc.vector.tensor_scalar_mul(sc2[:], sc2[:], ALPHA)

            # ---- Phase 2 ----
            with (
                tc.tile_pool(name="gtps", bufs=1, space="PSUM") as gtps_pool,
                tc.tile_pool(name="ops", bufs=3, space="PSUM") as ops_pool,
            ):
                for b in range(B):
                    gt2 = gt_pool.tile([128, SL], bf16, tag="gt")
                    for ib in range(NBLK):
                        gt_ps = gtps_pool.tile([128, 512], f32, tag="gtp")
                        nc.tensor.matmul(
                            gt_ps[:],
                            sc2[:],
                            vt_sb[0:NP, b, ib * 512:(ib + 1) * 512],
                            start=True, stop=True,
                        )
                        if ib == 0:
                            nc.vector.tensor_copy(
                                gt2[:, ib * 512:(ib + 1) * 512], gt_ps[:]
                            )
                        else:
                            nc.scalar.copy(
                                gt2[:, ib * 512:(ib + 1) * 512], gt_ps[:]
                            )
                    # stage-sorted groups of 3 chunks: all MMs, then the
                    # PSUM-draining copies/adds, then stores.  Copies never
                    # queue behind adds of earlier chunks, so PSUM banks
                    # recycle promptly and the PE stays fed.
                    for g0 in (0, 3, 6):
                        cs = list(range(g0, min(g0 + 3, NCH)))
                        pss = []
                        for c in cs:
                            o_ps = ops_pool.tile([128, SL], f32, tag="op")
                            lo = (c % 2) * 64
                            tp = {} if c % 2 == 0 else {
                                "tile_position": (64, 0)
                            }
                            for jb in range(NBLK):
                                sl_ = slice(jb * 512, (jb + 1) * 512)
                                nc.tensor.matmul(
                                    o_ps[:, sl_],
                                    gt2[lo:lo + NP, c * 128:(c + 1) * 128],
                                    vt_sb[lo:lo + NP, b, sl_],
                                    start=True, stop=True,
                                    **tp,
                                )
                            pss.append(o_ps)
                        ots = []
                        gsbs = []
                        for c, o_ps in zip(cs, pss):
                            m = (b * NCH + c) % 16
                            ot = o_pool.tile([128, SL], bf16, tag="o")
                            if m < 4:
                                gsbs.append(None)
                            else:
                                gsb = g_pool.tile([128, SL], bf16, tag="g")
                                nc.scalar.copy(gsb[:], o_ps[:])
                                gsbs.append(gsb)
                            ots.append(ot)
                        for c, o_ps, ot, gsb in zip(cs, pss, ots, gsbs):
                            m = (b * NCH + c) % 16
                            if gsb is None:
                                nc.vector.tensor_tensor(
                                    ot[:], s_sb[:, b, c, :], o_ps[:],
                                    mybir.AluOpType.add,
                                )
                            elif m < 14:
                                nc.vector.tensor_tensor(
                                    ot[:], s_sb[:, b, c, :], gsb[:],
                                    mybir.AluOpType.add,
                                )
                            else:
                                nc.gpsimd.tensor_tensor(
                                    ot[:], s_sb[:, b, c, :], gsb[:],
                                    mybir.AluOpType.add,
                                )
                        for c, ot in zip(cs, ots):
                            nc.sync.dma_start(
                                out_d[b, c * 128:(c + 1) * 128, :], ot[:]
                            )

    nc.compile()
    return nc


def _get_nc():
    if "nc" not in _CACHE:
        _CACHE["nc"] = _build_nc()
    return _CACHE["nc"]


def kernel(a_arc, s_arc, adds, pos, n_pos, _trace=False, _return_perf=False):
    from concourse.bass_utils import run_bass_kernel_spmd

    assert int(n_pos) == NP
    a = np.asarray(a_arc, dtype=np.float32)
    s = np.asarray(s_arc, dtype=np.float32)
    adds = np.asarray(adds)
    pos = np.asarray(pos)

    rng = np.arange(NP)
    eye = np.eye(NP, dtype=ml_dtypes.bfloat16)

    in_maps = []
    for k in range(NCORES):
        sl = slice(k * B, (k + 1) * B)
        adds_sh = adds[sl]
        pos_sh = pos[sl]
        # u[p, b, c, q] = [adds[b, c*128+p] == q]  (partition-major, fp8)
        u2 = (
            adds_sh.reshape(B, NCH, 128).transpose(2, 0, 1)[..., None] == rng
        ).astype(ml_dtypes.float8_e4m3)
        # vt2[p, b, i]: one-hot [pos==p] replicated at rows 0-49 and 64-113
        oh = (rng[:, None, None] == pos_sh[None, :, :]).astype(
            ml_dtypes.bfloat16
        )
        vt2 = np.zeros((128, B, SL), dtype=ml_dtypes.bfloat16)
        vt2[0:NP] = oh
        vt2[64:64 + NP] = oh
        in_maps.append(
            {
                "a": np.ascontiguousarray(a[sl]).astype(ml_dtypes.bfloat16),
                "s": np.ascontiguousarray(s[sl]).astype(ml_dtypes.bfloat16),
                "u": np.ascontiguousarray(u2),
                "vt": np.ascontiguousarray(vt2),
                "eye": eye,
            }
        )

    nc = _get_nc()
    res = run_bass_kernel_spmd(
        nc, in_maps, core_ids=list(range(NCORES)), trace=_trace
    )
    out = np.concatenate([r["out"] for r in res.results], axis=0).astype(np.float32)
    if _return_perf:
        return out, res
    return out
